# revision 1
# baseline (speedup 1.0000x reference)
"""Trainium2 Bass kernel for nn_MultiHeadAttention_7834020348049.

Reference computation (per token, no cross-token interaction):
    qn  = LayerNorm(q) * gamma_m + beta_m
    kvn = LayerNorm(kv) * gamma_l + beta_l
    Q = qn @ Wq.T ; K,V = split(kvn @ Wkv.T)
    per token: scores[h,g] = Q[h,:] . K[g,:] / sqrt(128)  (8x8 over heads)
    ctx[h,:] = softmax_g(scores) @ V
    out = ctx @ Wo.T

Sharding: pure data-parallel over the 16*2048 = 32768 tokens -> 4096/core.

Per-core pipeline (feature-major, fp32r matmuls):
  token-major LN (bn_stats/bn_aggr + tensor_scalar)
  -> PE transpose to feature-major qn^T / kvn^T
  -> projections with weights stationary (fp32r, N=256)
  -> per 16-token sub-tile: scores matmul S[(t,h),(t',g)] (128x128),
     exp on ACT, block-diag-masked tensor_tensor_reduce for softmax sums,
     P = E*mask/Z, PE transpose of P -> block-diag L, context matmul
     ctx^T = Vb^T-style (Vb = PE-transposed V slice), token-major O-proj.
"""
import sys, os
sys.path.insert(0, "/opt/trn_rl_repo")
os.environ.setdefault("JAX_PLATFORMS", "cpu")

from contextlib import ExitStack
import numpy as np

import concourse.bass as bass
import concourse.bacc as bacc
import concourse.tile as tile
from concourse import mybir
from concourse.masks import make_identity
from concourse.bass_utils import run_bass_kernel_spmd

F32 = mybir.dt.float32
F32R = mybir.dt.float32r

DIM = 1024
HEADS = 8
DHEAD = 128
NCORES = 8

# tokens per chunk (projection moving-dim; must be >=256 for fp32r full rate)
TC = 256
# tokens per tile (partition dim)
TT = 128
# tokens per attention sub-tile
TS = 16


def R(ap):
    return ap.bitcast(F32R)


def build_nc(T, with_bias_q=False, with_bias_kv=False):
    """Build the single-core Bass program for T tokens."""
    nc = bacc.Bacc(trn_type="TRN2", target_bir_lowering=False)

    q_d = nc.dram_tensor("q", [T, DIM], F32, kind="ExternalInput").ap()
    kv_d = nc.dram_tensor("kv", [T, DIM], F32, kind="ExternalInput").ap()
    wq_d = nc.dram_tensor("wq", [DIM, DIM], F32, kind="ExternalInput").ap()
    wkv_d = nc.dram_tensor("wkv", [DIM, 2 * DIM], F32, kind="ExternalInput").ap()
    wo_d = nc.dram_tensor("wo", [DIM, DIM], F32, kind="ExternalInput").ap()
    mask_d = nc.dram_tensor("mask", [TT, TT], F32, kind="ExternalInput").ap()
    bq_d = bkv_d = None
    if with_bias_q:
        bq_d = nc.dram_tensor("bq", [1, DIM], F32, kind="ExternalInput").ap()
    if with_bias_kv:
        bkv_d = nc.dram_tensor("bkv", [1, 2 * DIM], F32, kind="ExternalInput").ap()
    out_d = nc.dram_tensor("out", [T, DIM], F32, kind="ExternalOutput").ap()

    KT_F = DIM // TT          # 8 k-tiles for the 1024-feature contraction
    NCH = T // TC             # chunks
    TPC = TC // TT            # tiles per chunk (2)
    SPT = TT // TS            # sub-tiles per tile (8)

    with tile.TileContext(nc) as tc, ExitStack() as ctx:
        # ---------------- static SBUF ----------------
        singles = ctx.enter_context(tc.tile_pool(name="singles", bufs=1))
        ident = singles.tile([128, 128], F32)
        make_identity(nc, ident[:])
        mask = singles.tile([TT, TT], F32)
        nc.sync.dma_start(mask[:], mask_d)

        wq_sb = singles.tile([128, KT_F, DIM], F32)
        wkv_sb = singles.tile([128, KT_F, 2 * DIM], F32)
        wo_sb = singles.tile([128, KT_F, DIM], F32)
        for k in range(KT_F):
            nc.sync.dma_start(R(wq_sb[:, k, :]), R(wq_d[k * 128:(k + 1) * 128, :]))
            nc.sync.dma_start(R(wkv_sb[:, k, :]), R(wkv_d[k * 128:(k + 1) * 128, :]))
            nc.sync.dma_start(R(wo_sb[:, k, :]), R(wo_d[k * 128:(k + 1) * 128, :]))
        if with_bias_q:
            bq_sb = singles.tile([1, DIM], F32)
            nc.sync.dma_start(R(bq_sb[:]), R(bq_d))
            ones_row = singles.tile([1, TC], F32)
            nc.vector.memset(R(ones_row[:]), 1.0)
        if with_bias_kv:
            bkv_sb = singles.tile([1, 2 * DIM], F32)
            nc.sync.dma_start(R(bkv_sb[:]), R(bkv_d))
            if not with_bias_q:
                ones_row = singles.tile([1, TC], F32)
                nc.vector.memset(R(ones_row[:]), 1.0)

        # chunk-level feature-major activations (single-buffered)
        chunk_sb = ctx.enter_context(tc.tile_pool(name="chunk", bufs=1))
        qnT = chunk_sb.tile([128, KT_F, TC], F32, tag="qnT")
        kvnT = chunk_sb.tile([128, KT_F, TC], F32, tag="kvnT")
        # interleaved (t, h) column layout: col = t*HEADS + h
        QT = chunk_sb.tile([128, TC * HEADS], F32, tag="QT")
        KT = chunk_sb.tile([128, TC * HEADS], F32, tag="KT")
        VT = chunk_sb.tile([128, TC * HEADS], F32, tag="VT")

        # rotating pools
        raw_p = ctx.enter_context(tc.tile_pool(name="raw", bufs=2))
        st_p = ctx.enter_context(tc.tile_pool(name="stats", bufs=3))
        sm_p = ctx.enter_context(tc.tile_pool(name="smax", bufs=2))
        l_p = ctx.enter_context(tc.tile_pool(name="lbuf", bufs=2))
        vb_p = ctx.enter_context(tc.tile_pool(name="vbuf", bufs=2))
        ctxT_p = ctx.enter_context(tc.tile_pool(name="ctxT", bufs=1))
        outsb_p = ctx.enter_context(tc.tile_pool(name="outsb", bufs=2))

        ps_mm = ctx.enter_context(tc.tile_pool(name="ps_mm", bufs=2, space="PSUM"))
        ps_tr = ctx.enter_context(tc.tile_pool(name="ps_tr", bufs=2, space="PSUM"))
        ps_s = ctx.enter_context(tc.tile_pool(name="ps_s", bufs=2, space="PSUM"))
        ps_pt = ctx.enter_context(tc.tile_pool(name="ps_pt", bufs=1, space="PSUM"))
        ps_vc = ctx.enter_context(tc.tile_pool(name="ps_vc", bufs=1, space="PSUM"))

        def layernorm_tile(x):
            """in-place LN over free dim (1024) of x [128, 1024]."""
            stats = st_p.tile([128, 2, 6], F32, tag="bn")
            xg = x.rearrange("p (n f) -> p n f", n=2)
            for i in range(2):
                nc.vector.bn_stats(out=stats[:, i, :], in_=xg[:, i, :])
            mv = st_p.tile([128, 2], F32, tag="mv")
            nc.vector.bn_aggr(out=mv[:], in_=stats[:])
            eps = st_p.tile([128, 1], F32, tag="eps")
            nc.vector.memset(eps[:], 1e-5)
            rstd = st_p.tile([128, 1], F32, tag="rstd")
            nc.scalar.activation(out=rstd[:], in_=mv[:, 1:2],
                                 func=mybir.ActivationFunctionType.Sqrt,
                                 bias=eps[:], scale=1.0)
            nc.vector.reciprocal(out=rstd[:], in_=rstd[:])
            nc.vector.tensor_scalar(out=x, in0=x,
                                    scalar1=mv[:, 0:1],
                                    scalar2=rstd[:],
                                    op0=mybir.AluOpType.subtract,
                                    op1=mybir.AluOpType.mult)

        def head_cols(tens, m):
            """stride-HEADS columns of head m in an interleaved tensor."""
            return bass.AP(tensor=tens.tensor, offset=tens.offset + m,
                           ap=[tens.ap[0], [HEADS, TC]])

        for c in range(NCH):
            # ---------- stage A: load + LN + transpose to feature-major ----
            for it in range(TPC):
                tok0 = c * TC + it * TT
                for name, src, dstT in (("q", q_d, qnT), ("kv", kv_d, kvnT)):
                    x = raw_p.tile([128, DIM], F32, tag="raw")
                    nc.sync.dma_start(x[:], src[tok0:tok0 + TT, :])
                    layernorm_tile(x[:])
                    for f in range(KT_F):
                        tp = ps_tr.tile([128, 128], F32)
                        nc.tensor.transpose(
                            tp[:], x[:, f * 128:(f + 1) * 128], ident[:])
                        nc.scalar.copy(
                            out=R(dstT[:, f, it * TT:(it + 1) * TT]), in_=tp[:])

            # ---------- stage B: Q / KV projections (feature-major) --------
            for m in range(HEADS):
                ps = ps_mm.tile([128, TC], F32, tag="mm")
                for k in range(KT_F):
                    nc.tensor.matmul(
                        ps[:], R(wq_sb[:, k, m * 128:(m + 1) * 128]),
                        R(qnT[:, k, :]), start=(k == 0),
                        stop=(k == KT_F - 1 and not with_bias_q))
                if with_bias_q:
                    nc.tensor.matmul(
                        ps[:], R(bq_sb[:, m * 128:(m + 1) * 128]),
                        R(ones_row[:]), start=False, stop=True)
                nc.scalar.copy(out=head_cols(QT, m), in_=ps[:])
            for m in range(2 * HEADS):
                ps = ps_mm.tile([128, TC], F32, tag="mm")
                for k in range(KT_F):
                    nc.tensor.matmul(
                        ps[:], R(wkv_sb[:, k, m * 128:(m + 1) * 128]),
                        R(kvnT[:, k, :]), start=(k == 0),
                        stop=(k == KT_F - 1 and not with_bias_kv))
                if with_bias_kv:
                    nc.tensor.matmul(
                        ps[:], R(bkv_sb[:, m * 128:(m + 1) * 128]),
                        R(ones_row[:]), start=False, stop=True)
                dst = KT if m < HEADS else VT
                nc.scalar.copy(out=head_cols(dst, m % HEADS), in_=ps[:])

            # ---------- stage C: attention + O-projection per tile ---------
            for it in range(TPC):
                tok0 = c * TC + it * TT
                ctxT = ctxT_p.tile([128, HEADS, TT], F32, tag="ctxT")
                for s in range(SPT):
                    c0 = (it * TT + s * TS) * HEADS   # interleaved col base
                    # scores S[(t,h), (t',g)]
                    sps = ps_s.tile([128, 128], F32)
                    nc.tensor.matmul(sps[:], QT[:, c0:c0 + 128],
                                     KT[:, c0:c0 + 128],
                                     start=True, stop=True)
                    # E = exp(S)  (junk cross-token entries included, masked next)
                    e = sm_p.tile([128, 128], F32, tag="e")
                    nc.scalar.activation(out=e[:], in_=sps[:],
                                         func=mybir.ActivationFunctionType.Exp,
                                         scale=1.0)
                    # EM = E * mask ; Z = rowsum(EM)
                    em = sm_p.tile([128, 128], F32, tag="em")
                    z = st_p.tile([128, 1], F32, tag="z")
                    nc.vector.tensor_tensor(out=em[:], in0=e[:], in1=mask[:],
                                            op=mybir.AluOpType.mult)
                    nc.vector.tensor_reduce(out=z[:], in_=em[:],
                                            op=mybir.AluOpType.add,
                                            axis=mybir.AxisListType.X)
                    zr = st_p.tile([128, 1], F32, tag="zr")
                    nc.vector.reciprocal(out=zr[:], in_=z[:])
                    # P = EM / Z
                    p = sm_p.tile([128, 128], F32, tag="p")
                    nc.vector.tensor_scalar(out=p[:], in0=em[:],
                                            scalar1=zr[:], scalar2=None,
                                            op0=mybir.AluOpType.mult)
                    # L = P^T  (block-diagonal by construction)
                    ptp = ps_pt.tile([128, 128], F32)
                    nc.tensor.transpose(ptp[:], p[:], ident[:])
                    lbuf = l_p.tile([128, 128], F32, tag="l")
                    nc.vector.tensor_copy(out=lbuf[:], in_=ptp[:])
                    # Vb[(t,g), d] = transpose of V^T token-columns
                    vbp = ps_vc.tile([128, 128], F32, tag="vc")
                    nc.tensor.transpose(vbp[:], VT[:, c0:c0 + 128],
                                        ident[:])
                    vb = vb_p.tile([128, 128], F32, tag="vb")
                    nc.vector.tensor_copy(out=vb[:], in_=vbp[:])
                    # ctx^T[d, (t,h)] = Vb^T @ L
                    cps = ps_vc.tile([128, 128], F32, tag="vc")
                    nc.tensor.matmul(cps[:], vb[:], lbuf[:],
                                     start=True, stop=True)
                    ctx_dst = bass.AP(
                        tensor=ctxT.tensor,
                        offset=ctxT.offset + s * TS,
                        ap=[ctxT.ap[0], [1, TS], [TT, HEADS]])
                    nc.scalar.copy(out=R(ctx_dst), in_=cps[:])

                # O-projection, token-major: out[t, o] += ctxT_h^T @ wo_h
                for nn2 in range(2):
                    pso = ps_mm.tile([128, 512], F32, tag="mm")
                    for h in range(HEADS):
                        nc.tensor.matmul(
                            pso[:], R(ctxT[:, h, :]),
                            R(wo_sb[:, h, nn2 * 512:(nn2 + 1) * 512]),
                            start=(h == 0), stop=(h == HEADS - 1))
                    osb = outsb_p.tile([128, 512], F32, tag="osb")
                    nc.scalar.copy(out=osb[:], in_=pso[:])
                    nc.sync.dma_start(
                        out_d[tok0:tok0 + TT, nn2 * 512:(nn2 + 1) * 512], osb[:])

    nc.finalize()
    return nc


def _host_mask():
    m = np.zeros((TT, TT), np.float32)
    p = np.arange(TT)
    m[p[:, None] // HEADS == p[None, :] // HEADS] = 1.0
    return m


def kernel(q, kv, gamma_m, beta_m, gamma_l, beta_l, Wq, Wkv, Wo):
    q = np.asarray(q, np.float32)
    kv = np.asarray(kv, np.float32)
    bs, patch, _ = q.shape
    T_total = bs * patch
    T_core = T_total // NCORES

    scale = DHEAD ** (-0.5)
    # fold LN gamma into the projection weights, beta into bias vectors
    wq_eff = (np.asarray(Wq, np.float32) * np.asarray(gamma_m, np.float32)[None, :]) * scale
    bq = (np.asarray(Wq, np.float32) @ np.asarray(beta_m, np.float32)) * scale
    wkv_eff = np.asarray(Wkv, np.float32) * np.asarray(gamma_l, np.float32)[None, :]
    bkv = np.asarray(Wkv, np.float32) @ np.asarray(beta_l, np.float32)
    with_bias_q = bool(np.any(bq != 0.0))
    with_bias_kv = bool(np.any(bkv != 0.0))

    # kernel weight layout: [in, out]
    wq_t = np.ascontiguousarray(wq_eff.T)
    wkv_t = np.ascontiguousarray(wkv_eff.T)
    wo_t = np.ascontiguousarray(np.asarray(Wo, np.float32).T)
    mask = _host_mask()

    nc = build_nc(T_core, with_bias_q, with_bias_kv)

    qf = q.reshape(T_total, DIM)
    kvf = kv.reshape(T_total, DIM)
    in_maps = []
    for i in range(NCORES):
        m = {
            "q": np.ascontiguousarray(qf[i * T_core:(i + 1) * T_core]),
            "kv": np.ascontiguousarray(kvf[i * T_core:(i + 1) * T_core]),
            "wq": wq_t, "wkv": wkv_t, "wo": wo_t, "mask": mask,
        }
        if with_bias_q:
            m["bq"] = bq.reshape(1, DIM).astype(np.float32)
        if with_bias_kv:
            m["bkv"] = bkv.reshape(1, 2 * DIM).astype(np.float32)
        in_maps.append(m)

    res = run_bass_kernel_spmd(nc, in_maps, list(range(NCORES)))
    global LAST_RESULTS
    LAST_RESULTS = res
    out = np.concatenate([res.results[i]["out"] for i in range(NCORES)], axis=0)
    return out.reshape(bs, patch, DIM)


LAST_RESULTS = None



# revision 13
# speedup vs baseline: 1.3395x; 1.3395x over previous
"""Trainium2 Bass kernel for nn_MultiHeadAttention_7834020348049.

Reference computation (per token, no cross-token interaction):
    qn  = LayerNorm(q) * gamma_m + beta_m
    kvn = LayerNorm(kv) * gamma_l + beta_l
    Q = qn @ Wq.T ; K,V = split(kvn @ Wkv.T)
    per token: scores[h,g] = Q[h,:] . K[g,:] / sqrt(128)  (8x8 over heads)
    ctx[h,:] = softmax_g(scores) @ V
    out = ctx @ Wo.T

Sharding: pure data-parallel over the 16*2048 = 32768 tokens -> 4096/core.

v2 pipeline (all-fp16 matmuls at 1 cycle/row, bf16 softmax for range):
  token-major LN (bn_stats/bn_aggr fp32 stats, fp16 data)
  -> PE transpose (fp16) to feature-major qn^T / kvn^T
  -> projections with weights stationary, 512-token chunks, per-head
     contiguous Q^T/K^T/V^T [d][h][t] (strided matmul-operand APs instead
     of strided writes)
  -> per 128-token tile, two 4-subtile batches:
     scores S[(t,h),(t,g)] via strided APs, batched exp (ACT, bf16),
     tensor_tensor_reduce mask+rowsum, reciprocal, tensor_scalar P (fp16),
     PE transposes of P and V (fp16 PSUM), ctx matmul, batched copies
  -> token-major O-projection (strided ctx^T head reads), fp16 output.
"""
import sys, os
sys.path.insert(0, "/opt/trn_rl_repo")
os.environ.setdefault("JAX_PLATFORMS", "cpu")

from contextlib import ExitStack
import numpy as np
import ml_dtypes

import concourse.bass as bass
import concourse.bacc as bacc
import concourse.tile as tile
from concourse import mybir
from concourse.masks import make_identity
from concourse.bass_utils import run_bass_kernel_spmd

F32 = mybir.dt.float32
F16 = mybir.dt.float16
BF16 = mybir.dt.bfloat16

DIM = 1024
HEADS = 8
DHEAD = 128
NCORES = 8

TC = 512   # tokens per chunk (projection moving-dim)
TT = 128   # tokens per tile (partition dim)
TS = 16    # tokens per attention sub-tile
KT_F = DIM // 128  # 8 k-tiles for the 1024-feature contraction


def head_strided(t, h, n):
    """stride-HEADS columns of head h in a (t,h)-interleaved tensor:
    col i = token, addr = i*HEADS + h. Single free dim (matmul-legal)."""
    return bass.AP(tensor=t.tensor, offset=t.offset + h,
                   ap=[t.ap[0], [HEADS, n]])


def build_nc(T, with_bias_q=False, with_bias_kv=False):
    nc = bacc.Bacc(trn_type="TRN2", target_bir_lowering=False)

    q_d = nc.dram_tensor("q", [T, DIM], F16, kind="ExternalInput").ap()
    kv_d = nc.dram_tensor("kv", [T, DIM], F16, kind="ExternalInput").ap()
    wq_d = nc.dram_tensor("wq", [DIM, DIM], F16, kind="ExternalInput").ap()
    wkv_d = nc.dram_tensor("wkv", [DIM, 2 * DIM], F16, kind="ExternalInput").ap()
    wo_d = nc.dram_tensor("wo", [DIM, DIM], F16, kind="ExternalInput").ap()
    mask_d = nc.dram_tensor("mask", [TT, TT], BF16, kind="ExternalInput").ap()
    bq_d = bkv_d = None
    if with_bias_q:
        bq_d = nc.dram_tensor("bq", [1, DIM], F16, kind="ExternalInput").ap()
    if with_bias_kv:
        bkv_d = nc.dram_tensor("bkv", [1, 2 * DIM], F16, kind="ExternalInput").ap()
    out_d = nc.dram_tensor("out", [T, DIM], F16, kind="ExternalOutput").ap()

    NCH = T // TC        # chunks
    TPC = TC // TT       # tiles per chunk (4)
    SPT = TT // TS       # sub-tiles per tile (8)

    with tile.TileContext(nc) as tc, ExitStack() as ctx:
        # ---------------- static SBUF ----------------
        singles = ctx.enter_context(tc.tile_pool(name="singles", bufs=1))
        ident = singles.tile([128, 128], F16)
        make_identity(nc, ident[:])
        mask = singles.tile([TT, TT], BF16)
        nc.sync.dma_start(mask[:], mask_d)
        eps = singles.tile([128, 1], F32)
        nc.vector.memset(eps[:], 1e-5)

        wq_sb = singles.tile([128, KT_F, DIM], F16)
        wkv_sb = singles.tile([128, KT_F, 2 * DIM], F16)
        wo_sb = singles.tile([128, KT_F, DIM], F16)
        for k in range(KT_F):
            nc.sync.dma_start(wq_sb[:, k, :], wq_d[k * 128:(k + 1) * 128, :])
            nc.sync.dma_start(wkv_sb[:, k, :], wkv_d[k * 128:(k + 1) * 128, :])
            nc.sync.dma_start(wo_sb[:, k, :], wo_d[k * 128:(k + 1) * 128, :])
        if with_bias_q or with_bias_kv:
            ones_row = singles.tile([1, TC], F16)
            nc.vector.memset(ones_row[:], 1.0)
        if with_bias_q:
            bq_sb = singles.tile([1, DIM], F16)
            nc.sync.dma_start(bq_sb[:], bq_d)
        if with_bias_kv:
            bkv_sb = singles.tile([1, 2 * DIM], F16)
            nc.sync.dma_start(bkv_sb[:], bkv_d)

        # chunk-level feature-major activations (single-buffered)
        chunk_sb = ctx.enter_context(tc.tile_pool(name="chunk", bufs=1))
        qnT = chunk_sb.tile([128, KT_F, TC], F16, tag="qnT")
        kvnT = chunk_sb.tile([128, KT_F, TC], F16, tag="kvnT")
        # (t,h)-interleaved columns: col = t*HEADS + h
        QT = chunk_sb.tile([128, TC * HEADS], F16, tag="QT")
        KT = chunk_sb.tile([128, TC * HEADS], F16, tag="KT")
        VT = chunk_sb.tile([128, TC * HEADS], F16, tag="VT")

        raw_p = ctx.enter_context(tc.tile_pool(name="raw", bufs=16))
        st_p = ctx.enter_context(tc.tile_pool(name="stats", bufs=6))
        e_p = ctx.enter_context(tc.tile_pool(name="ebuf", bufs=2))
        em_p = ctx.enter_context(tc.tile_pool(name="embuf", bufs=2))
        p_p = ctx.enter_context(tc.tile_pool(name="pbuf", bufs=2))
        z_p = ctx.enter_context(tc.tile_pool(name="zbuf", bufs=4))
        l_p = ctx.enter_context(tc.tile_pool(name="lbuf", bufs=2))
        vb_p = ctx.enter_context(tc.tile_pool(name="vbuf", bufs=2))
        ctxT_p = ctx.enter_context(tc.tile_pool(name="ctxT", bufs=2))
        outsb_p = ctx.enter_context(tc.tile_pool(name="outsb", bufs=4))

        ps_tr = ctx.enter_context(tc.tile_pool(name="ps_tr", bufs=2, space="PSUM"))
        ps_mm = ctx.enter_context(tc.tile_pool(name="ps_mm", bufs=2, space="PSUM"))
        ps_s = ctx.enter_context(tc.tile_pool(name="ps_s", bufs=2, space="PSUM"))
        ps_o = ctx.enter_context(tc.tile_pool(name="ps_o", bufs=2, space="PSUM"))

        # per-chunk raw tiles, alive across two chunks (LN in chunk c-1's
        # emission, transposed at chunk c)
        raw_tiles = {}

        def emit_loads(c):
            if c >= NCH:
                return
            for it in range(TPC):
                tok0 = c * TC + it * TT
                for name, src in (("q", q_d), ("kv", kv_d)):
                    x = raw_p.tile([128, DIM], F16, tag="raw")
                    nc.sync.dma_start(x[:], src[tok0:tok0 + TT, :])
                    raw_tiles[(c, it, name)] = x

        def emit_ln(c, it):
            """LayerNorm both tensors of tile (c, it) in place."""
            if c >= NCH:
                return
            for name in ("q", "kv"):
                x = raw_tiles[(c, it, name)]
                stats = st_p.tile([128, 2, 6], F32, tag="bn")
                xg = x.rearrange("p (n f) -> p n f", n=2)
                for i in range(2):
                    nc.vector.bn_stats(out=stats[:, i, :], in_=xg[:, i, :])
                mv = st_p.tile([128, 2], F32, tag="mv")
                nc.vector.bn_aggr(out=mv[:], in_=stats[:])
                rstd = st_p.tile([128, 1], F32, tag="rstd")
                nc.scalar.activation(out=rstd[:], in_=mv[:, 1:2],
                                     func=mybir.ActivationFunctionType.Sqrt,
                                     bias=eps[:], scale=1.0)
                nc.vector.reciprocal(out=rstd[:], in_=rstd[:])
                nc.vector.tensor_scalar(out=x[:], in0=x[:],
                                        scalar1=mv[:, 0:1],
                                        scalar2=rstd[:],
                                        op0=mybir.AluOpType.subtract,
                                        op1=mybir.AluOpType.mult)

        def emit_transposes(c):
            """PE transpose LN'd tiles to feature-major qnT/kvnT."""
            for it in range(TPC):
                for name, dstT in (("q", qnT), ("kv", kvnT)):
                    x = raw_tiles.pop((c, it, name))
                    for fb in range(2):
                        tp = ps_tr.tile([128, 4, 128], F16, tag="tr")
                        for f4 in range(4):
                            f = fb * 4 + f4
                            nc.tensor.transpose(
                                tp[:, f4, :], x[:, f * 128:(f + 1) * 128],
                                ident[:], )
                        nc.vector.tensor_copy(
                            out=dstT[:, fb * 4:(fb + 1) * 4,
                                     it * TT:(it + 1) * TT],
                            in_=tp[:])

        def emit_projections(c):
            for m in range(HEADS):
                ps = ps_mm.tile([128, TC], F32, tag="mm")
                for k in range(KT_F):
                    nc.tensor.matmul(
                        ps[:], wq_sb[:, k, m * 128:(m + 1) * 128],
                        qnT[:, k, :], start=(k == 0),
                        stop=(k == KT_F - 1 and not with_bias_q))
                if with_bias_q:
                    nc.tensor.matmul(
                        ps[:], bq_sb[:, m * 128:(m + 1) * 128],
                        ones_row[:], start=False, stop=True)
                nc.scalar.copy(out=head_strided(QT, m, TC), in_=ps[:])
            for m in range(2 * HEADS):
                ps = ps_mm.tile([128, TC], F32, tag="mm")
                for k in range(KT_F):
                    nc.tensor.matmul(
                        ps[:], wkv_sb[:, k, m * 128:(m + 1) * 128],
                        kvnT[:, k, :], start=(k == 0),
                        stop=(k == KT_F - 1 and not with_bias_kv))
                if with_bias_kv:
                    nc.tensor.matmul(
                        ps[:], bkv_sb[:, m * 128:(m + 1) * 128],
                        ones_row[:], start=False, stop=True)
                dst = KT if m < HEADS else VT
                nc.scalar.copy(out=head_strided(dst, m % HEADS, TC), in_=ps[:])

        def emit_attention_tile(c, it):
            """Attention for tile it; returns ctxT tile for the O-proj."""
            t0 = it * TT
            E = e_p.tile([128, SPT, 128], BF16, tag="e")
            EM = em_p.tile([128, SPT, 128], BF16, tag="em")
            P = p_p.tile([128, SPT, 128], F16, tag="p")
            z = z_p.tile([128, SPT], F32, tag="z")
            zr = z_p.tile([128, SPT], F32, tag="zr")
            L = l_p.tile([128, SPT, 128], F16, tag="l")
            Vb = vb_p.tile([128, SPT, 128], F16, tag="vb")
            ctxT = ctxT_p.tile([128, SPT * 128], F16, tag="ctxT")

            for b in range(2):
                # scores for 4 sub-tiles into one PSUM bank
                sps = ps_s.tile([128, 4, 128], F32, tag="s")
                for s4 in range(4):
                    c0 = (t0 + (b * 4 + s4) * TS) * HEADS
                    nc.tensor.matmul(
                        sps[:, s4, :],
                        QT[:, c0:c0 + 128], KT[:, c0:c0 + 128],
                        start=True, stop=True, skip_group_check=True)
                # V^T sub-blocks -> token-major Vb (independent of scores)
                vtp = ps_tr.tile([128, 4, 128], F16, tag="tr")
                for s4 in range(4):
                    c0 = (t0 + (b * 4 + s4) * TS) * HEADS
                    nc.tensor.transpose(
                        vtp[:, s4, :], VT[:, c0:c0 + 128], ident[:])
                nc.vector.tensor_copy(
                    out=Vb[:, b * 4:(b + 1) * 4, :], in_=vtp[:])
                # E = exp(S) batched (bf16 for range)
                nc.scalar.activation(
                    out=E[:, b * 4:(b + 1) * 4, :], in_=sps[:],
                    func=mybir.ActivationFunctionType.Exp, scale=1.0)
                # EM = E*mask ; z = rowsum(EM)
                for s4 in range(4):
                    s = b * 4 + s4
                    nc.vector.tensor_tensor(
                        out=EM[:, s, :], in0=E[:, s, :], in1=mask[:],
                        op=mybir.AluOpType.mult)
                nc.vector.tensor_reduce(
                    out=z[:, b * 4:(b + 1) * 4],
                    in_=EM[:, b * 4:(b + 1) * 4, :],
                    op=mybir.AluOpType.add, axis=mybir.AxisListType.X)
                nc.vector.reciprocal(out=zr[:, b * 4:(b + 1) * 4],
                                     in_=z[:, b * 4:(b + 1) * 4])
                for s4 in range(4):
                    s = b * 4 + s4
                    nc.vector.tensor_scalar(
                        out=P[:, s, :], in0=EM[:, s, :],
                        scalar1=zr[:, s:s + 1], scalar2=None,
                        op0=mybir.AluOpType.mult)
                # L = P^T (block-diagonal)
                ptp = ps_tr.tile([128, 4, 128], F16, tag="tr")
                for s4 in range(4):
                    s = b * 4 + s4
                    nc.tensor.transpose(ptp[:, s4, :], P[:, s, :], ident[:])
                nc.vector.tensor_copy(
                    out=L[:, b * 4:(b + 1) * 4, :], in_=ptp[:])
                # ctx^T[d, (t,h)] = Vb^T @ L per sub-tile
                cps = ps_s.tile([128, 4, 128], F32, tag="s")
                for s4 in range(4):
                    s = b * 4 + s4
                    nc.tensor.matmul(
                        cps[:, s4, :], Vb[:, s, :], L[:, s, :],
                        start=True, stop=True, skip_group_check=True)
                nc.scalar.copy(out=ctxT[:, b * 512:(b + 1) * 512], in_=cps[:])
            return ctxT

        def emit_oproj(c, it, ctxT):
            tok0 = c * TC + it * TT
            for oh in range(2):
                pso = ps_o.tile([128, 512], F32, tag="o")
                for h in range(HEADS):
                    nc.tensor.matmul(
                        pso[:], head_strided(ctxT, h, TT),
                        wo_sb[:, h, oh * 512:(oh + 1) * 512],
                        start=(h == 0), stop=(h == HEADS - 1))
                osb = outsb_p.tile([128, 512], F16, tag="osb")
                nc.scalar.copy(out=osb[:], in_=pso[:])
                nc.sync.dma_start(
                    out_d[tok0:tok0 + TT, oh * 512:(oh + 1) * 512], osb[:])

        # ---------------- main schedule ----------------
        emit_loads(0)
        emit_loads(1)
        for it in range(TPC):
            emit_ln(0, it)

        for c in range(NCH):
            emit_loads(c + 2)
            emit_transposes(c)
            emit_projections(c)
            ctx_tiles = {}
            # stage C with next-chunk LN interleaved on DVE; O-proj lags
            # one tile so PE never waits on the ACT ctx copies.
            ctx_tiles[0] = emit_attention_tile(c, 0)
            emit_ln(c + 1, 0)
            ctx_tiles[1] = emit_attention_tile(c, 1)
            emit_ln(c + 1, 1)
            emit_oproj(c, 0, ctx_tiles.pop(0))
            ctx_tiles[2] = emit_attention_tile(c, 2)
            emit_ln(c + 1, 2)
            emit_oproj(c, 1, ctx_tiles.pop(1))
            ctx_tiles[3] = emit_attention_tile(c, 3)
            emit_ln(c + 1, 3)
            emit_oproj(c, 2, ctx_tiles.pop(2))
            emit_oproj(c, 3, ctx_tiles.pop(3))

    nc.finalize()
    return nc


def _host_mask():
    m = np.zeros((TT, TT), np.float32)
    p = np.arange(TT)
    m[p[:, None] // HEADS == p[None, :] // HEADS] = 1.0
    return m.astype(ml_dtypes.bfloat16)


def kernel(q, kv, gamma_m, beta_m, gamma_l, beta_l, Wq, Wkv, Wo):
    q = np.asarray(q, np.float32)
    kv = np.asarray(kv, np.float32)
    bs, patch, _ = q.shape
    T_total = bs * patch
    T_core = T_total // NCORES

    scale = DHEAD ** (-0.5)
    # fold LN gamma into the projection weights, beta into bias vectors
    wq_eff = (np.asarray(Wq, np.float32) * np.asarray(gamma_m, np.float32)[None, :]) * scale
    bq = (np.asarray(Wq, np.float32) @ np.asarray(beta_m, np.float32)) * scale
    wkv_eff = np.asarray(Wkv, np.float32) * np.asarray(gamma_l, np.float32)[None, :]
    bkv = np.asarray(Wkv, np.float32) @ np.asarray(beta_l, np.float32)
    with_bias_q = bool(np.any(bq != 0.0))
    with_bias_kv = bool(np.any(bkv != 0.0))

    # kernel weight layout: [in, out], fp16
    wq_t = np.ascontiguousarray(wq_eff.T).astype(np.float16)
    wkv_t = np.ascontiguousarray(wkv_eff.T).astype(np.float16)
    wo_t = np.ascontiguousarray(np.asarray(Wo, np.float32).T).astype(np.float16)
    mask = _host_mask()

    nc = build_nc(T_core, with_bias_q, with_bias_kv)

    qf = q.reshape(T_total, DIM).astype(np.float16)
    kvf = kv.reshape(T_total, DIM).astype(np.float16)
    in_maps = []
    for i in range(NCORES):
        m = {
            "q": np.ascontiguousarray(qf[i * T_core:(i + 1) * T_core]),
            "kv": np.ascontiguousarray(kvf[i * T_core:(i + 1) * T_core]),
            "wq": wq_t, "wkv": wkv_t, "wo": wo_t, "mask": mask,
        }
        if with_bias_q:
            m["bq"] = bq.reshape(1, DIM).astype(np.float16)
        if with_bias_kv:
            m["bkv"] = bkv.reshape(1, 2 * DIM).astype(np.float16)
        in_maps.append(m)

    res = run_bass_kernel_spmd(nc, in_maps, list(range(NCORES)))
    global LAST_RESULTS
    LAST_RESULTS = res
    out = np.concatenate(
        [np.asarray(res.results[i]["out"], np.float32) for i in range(NCORES)],
        axis=0)
    return out.reshape(bs, patch, DIM)


LAST_RESULTS = None


# revision 22
# speedup vs baseline: 1.5452x; 1.1536x over previous
"""Trainium2 Bass kernel for nn_MultiHeadAttention_7834020348049.

Reference computation (per token, no cross-token interaction):
    qn  = LayerNorm(q) * gamma_m + beta_m
    kvn = LayerNorm(kv) * gamma_l + beta_l
    Q = qn @ Wq.T ; K,V = split(kvn @ Wkv.T)
    per token: scores[h,g] = Q[h,:] . K[g,:] / sqrt(128)  (8x8 over heads)
    ctx[h,:] = softmax_g(scores) @ V
    out = ctx @ Wo.T

Sharding: pure data-parallel over the 16*2048 = 32768 tokens -> 4096/core.

v2 pipeline (all-fp16 matmuls at 1 cycle/row, bf16 softmax for range):
  token-major LN (bn_stats/bn_aggr fp32 stats, fp16 data)
  -> PE transpose (fp16) to feature-major qn^T / kvn^T
  -> projections with weights stationary, 512-token chunks, per-head
     contiguous Q^T/K^T/V^T [d][h][t] (strided matmul-operand APs instead
     of strided writes)
  -> per 128-token tile, two 4-subtile batches:
     scores S[(t,h),(t,g)] via strided APs, batched exp (ACT, bf16),
     tensor_tensor_reduce mask+rowsum, reciprocal, tensor_scalar P (fp16),
     PE transposes of P and V (fp16 PSUM), ctx matmul, batched copies
  -> token-major O-projection (strided ctx^T head reads), fp16 output.
"""
import sys, os
sys.path.insert(0, "/opt/trn_rl_repo")
os.environ.setdefault("JAX_PLATFORMS", "cpu")

from contextlib import ExitStack
import numpy as np
import ml_dtypes

import concourse.bass as bass
import concourse.bacc as bacc
import concourse.tile as tile
from concourse import mybir
from concourse.masks import make_identity
from concourse.bass_utils import run_bass_kernel_spmd

F32 = mybir.dt.float32
F16 = mybir.dt.float16
BF16 = mybir.dt.bfloat16

DIM = 1024
HEADS = 8
DHEAD = 128
NCORES = 8

TC = 512   # tokens per chunk (projection moving-dim)
TT = 128   # tokens per tile (partition dim)
TS = 16    # tokens per attention sub-tile
KT_F = DIM // 128  # 8 k-tiles for the 1024-feature contraction


def head_windows(t, h, nwin):
    """head-h columns across nwin h-major sub-tile windows of an
    interleaved [128, nwin*128] tensor: window w holds cols
    w*128 + h*16 + t_local. Two free dims, 16-elem packed runs."""
    return bass.AP(tensor=t.tensor, offset=t.offset + h * TS,
                   ap=[t.ap[0], [128, nwin], [1, TS]])


def build_nc(T, with_bias_q=False, with_bias_kv=False):
    nc = bacc.Bacc(trn_type="TRN2", target_bir_lowering=False)

    q_d = nc.dram_tensor("q", [T, DIM], F16, kind="ExternalInput").ap()
    kv_d = nc.dram_tensor("kv", [T, DIM], F16, kind="ExternalInput").ap()
    wq_d = nc.dram_tensor("wq", [DIM, DIM], F16, kind="ExternalInput").ap()
    wkv_d = nc.dram_tensor("wkv", [DIM, 2 * DIM], F16, kind="ExternalInput").ap()
    wo_d = nc.dram_tensor("wo", [DIM, DIM], F16, kind="ExternalInput").ap()
    mask_d = nc.dram_tensor("mask", [TT, TT], BF16, kind="ExternalInput").ap()
    bq_d = bkv_d = None
    if with_bias_q:
        bq_d = nc.dram_tensor("bq", [1, DIM], F16, kind="ExternalInput").ap()
    if with_bias_kv:
        bkv_d = nc.dram_tensor("bkv", [1, 2 * DIM], F16, kind="ExternalInput").ap()
    out_d = nc.dram_tensor("out", [T, DIM], F16, kind="ExternalOutput").ap()

    NCH = T // TC        # chunks
    TPC = TC // TT       # tiles per chunk (4)
    SPT = TT // TS       # sub-tiles per tile (8)

    with tile.TileContext(nc) as tc, ExitStack() as ctx:
        # ---------------- static SBUF ----------------
        singles = ctx.enter_context(tc.tile_pool(name="singles", bufs=1))
        ident = singles.tile([128, 128], F16)
        make_identity(nc, ident[:])
        mask = singles.tile([TT, TT], BF16)
        nc.sync.dma_start(mask[:], mask_d)
        eps = singles.tile([128, 1], F32)
        nc.vector.memset(eps[:], 1e-5)

        wq_sb = singles.tile([128, KT_F, DIM], F16)
        wkv_sb = singles.tile([128, KT_F, 2 * DIM], F16)
        wo_sb = singles.tile([128, KT_F, DIM], F16)
        for k in range(KT_F):
            nc.sync.dma_start(wq_sb[:, k, :], wq_d[k * 128:(k + 1) * 128, :])
            nc.sync.dma_start(wkv_sb[:, k, :], wkv_d[k * 128:(k + 1) * 128, :])
            nc.sync.dma_start(wo_sb[:, k, :], wo_d[k * 128:(k + 1) * 128, :])
        if with_bias_q or with_bias_kv:
            ones_row = singles.tile([1, TC], F16)
            nc.vector.memset(ones_row[:], 1.0)
        if with_bias_q:
            bq_sb = singles.tile([1, DIM], F16)
            nc.sync.dma_start(bq_sb[:], bq_d)
        if with_bias_kv:
            bkv_sb = singles.tile([1, 2 * DIM], F16)
            nc.sync.dma_start(bkv_sb[:], bkv_d)

        # chunk-level feature-major activations (single-buffered)
        chunk_sb = ctx.enter_context(tc.tile_pool(name="chunk", bufs=1))
        qnT = chunk_sb.tile([128, KT_F, TC], F16, tag="qnT")
        kvnT = chunk_sb.tile([128, KT_F, TC], F16, tag="kvnT")
        # h-major sub-tile windows: window w (16 tokens), col = w*128 + h*16 + t
        QT = chunk_sb.tile([128, TC * HEADS], F16, tag="QT")
        KT = chunk_sb.tile([128, TC * HEADS], F16, tag="KT")
        VT = chunk_sb.tile([128, TC * HEADS], F16, tag="VT")

        raw_p = ctx.enter_context(tc.tile_pool(name="raw", bufs=16))
        st_p = ctx.enter_context(tc.tile_pool(name="stats", bufs=6))
        e_p = ctx.enter_context(tc.tile_pool(name="ebuf", bufs=2))
        em_p = ctx.enter_context(tc.tile_pool(name="embuf", bufs=2))
        p_p = ctx.enter_context(tc.tile_pool(name="pbuf", bufs=2))
        z_p = ctx.enter_context(tc.tile_pool(name="zbuf", bufs=4))
        l_p = ctx.enter_context(tc.tile_pool(name="lbuf", bufs=2))
        vb_p = ctx.enter_context(tc.tile_pool(name="vbuf", bufs=2))
        ctxT_p = ctx.enter_context(tc.tile_pool(name="ctxT", bufs=2))
        outsb_p = ctx.enter_context(tc.tile_pool(name="outsb", bufs=4))

        ps_tr = ctx.enter_context(tc.tile_pool(name="ps_tr", bufs=2, space="PSUM"))
        ps_mm = ctx.enter_context(tc.tile_pool(name="ps_mm", bufs=2, space="PSUM"))
        ps_s = ctx.enter_context(tc.tile_pool(name="ps_s", bufs=2, space="PSUM"))
        ps_o = ctx.enter_context(tc.tile_pool(name="ps_o", bufs=2, space="PSUM"))

        # per-chunk raw tiles, alive across two chunks (LN in chunk c-1's
        # emission, transposed at chunk c)
        raw_tiles = {}

        def emit_loads(c):
            if c >= NCH:
                return
            for it in range(TPC):
                tok0 = c * TC + it * TT
                for name, src in (("q", q_d), ("kv", kv_d)):
                    x = raw_p.tile([128, DIM], F16, tag="raw")
                    nc.sync.dma_start(x[:], src[tok0:tok0 + TT, :])
                    raw_tiles[(c, it, name)] = x

        def emit_ln(c, it):
            """LayerNorm both tensors of tile (c, it) in place."""
            if c >= NCH:
                return
            for name in ("q", "kv"):
                x = raw_tiles[(c, it, name)]
                stats = st_p.tile([128, 2, 6], F32, tag="bn")
                xg = x.rearrange("p (n f) -> p n f", n=2)
                for i in range(2):
                    nc.vector.bn_stats(out=stats[:, i, :], in_=xg[:, i, :])
                mv = st_p.tile([128, 2], F32, tag="mv")
                nc.vector.bn_aggr(out=mv[:], in_=stats[:])
                rstd = st_p.tile([128, 1], F32, tag="rstd")
                nc.scalar.activation(out=rstd[:], in_=mv[:, 1:2],
                                     func=mybir.ActivationFunctionType.Sqrt,
                                     bias=eps[:], scale=1.0)
                nc.vector.reciprocal(out=rstd[:], in_=rstd[:])
                nc.vector.tensor_scalar(out=x[:], in0=x[:],
                                        scalar1=mv[:, 0:1],
                                        scalar2=rstd[:],
                                        op0=mybir.AluOpType.subtract,
                                        op1=mybir.AluOpType.mult)

        def emit_transposes(c):
            """PE transpose LN'd tiles to feature-major qnT/kvnT."""
            for it in range(TPC):
                for name, dstT in (("q", qnT), ("kv", kvnT)):
                    x = raw_tiles.pop((c, it, name))
                    for fb in range(2):
                        tp = ps_tr.tile([128, 4, 128], F16, tag="tr")
                        for f4 in range(4):
                            f = fb * 4 + f4
                            nc.tensor.transpose(
                                tp[:, f4, :], x[:, f * 128:(f + 1) * 128],
                                ident[:], )
                        nc.vector.tensor_copy(
                            out=dstT[:, fb * 4:(fb + 1) * 4,
                                     it * TT:(it + 1) * TT],
                            in_=tp[:])

        def emit_projections(c):
            for m in range(HEADS):
                ps = ps_mm.tile([128, TC], F32, tag="mm")
                for k in range(KT_F):
                    nc.tensor.matmul(
                        ps[:], wq_sb[:, k, m * 128:(m + 1) * 128],
                        qnT[:, k, :], start=(k == 0),
                        stop=(k == KT_F - 1 and not with_bias_q))
                if with_bias_q:
                    nc.tensor.matmul(
                        ps[:], bq_sb[:, m * 128:(m + 1) * 128],
                        ones_row[:], start=False, stop=True)
                nc.scalar.copy(out=head_windows(QT, m, TC // TS), in_=ps[:])
            for m in range(2 * HEADS):
                ps = ps_mm.tile([128, TC], F32, tag="mm")
                for k in range(KT_F):
                    nc.tensor.matmul(
                        ps[:], wkv_sb[:, k, m * 128:(m + 1) * 128],
                        kvnT[:, k, :], start=(k == 0),
                        stop=(k == KT_F - 1 and not with_bias_kv))
                if with_bias_kv:
                    nc.tensor.matmul(
                        ps[:], bkv_sb[:, m * 128:(m + 1) * 128],
                        ones_row[:], start=False, stop=True)
                dst = KT if m < HEADS else VT
                nc.scalar.copy(out=head_windows(dst, m % HEADS, TC // TS),
                               in_=ps[:])

        def emit_attention_tile(c, it):
            """Attention for tile it; returns ctxT tile for the O-proj."""
            t0 = it * TT
            E = e_p.tile([128, SPT, 128], BF16, tag="e")
            EM = em_p.tile([128, SPT, 128], BF16, tag="em")
            P = p_p.tile([128, SPT, 128], F16, tag="p")
            z = z_p.tile([128, SPT], F32, tag="z")
            zr = z_p.tile([128, SPT], F32, tag="zr")
            L = l_p.tile([128, SPT, 128], F16, tag="l")
            Vb = vb_p.tile([128, SPT, 128], F16, tag="vb")
            ctxT = ctxT_p.tile([128, HEADS, TT], F16, tag="ctxT")

            for b in range(2):
                # scores for 4 sub-tiles into one PSUM bank
                sps = ps_s.tile([128, 4, 128], F32, tag="s")
                for s4 in range(4):
                    c0 = (t0 + (b * 4 + s4) * TS) * HEADS
                    nc.tensor.matmul(
                        sps[:, s4, :],
                        QT[:, c0:c0 + 128], KT[:, c0:c0 + 128],
                        start=True, stop=True, skip_group_check=True)
                # V^T sub-blocks -> token-major Vb (independent of scores)
                vtp = ps_tr.tile([128, 4, 128], F16, tag="tr")
                for s4 in range(4):
                    c0 = (t0 + (b * 4 + s4) * TS) * HEADS
                    nc.tensor.transpose(
                        vtp[:, s4, :], VT[:, c0:c0 + 128], ident[:])
                nc.vector.tensor_copy(
                    out=Vb[:, b * 4:(b + 1) * 4, :], in_=vtp[:])
                # E = exp(S) batched (bf16 for range)
                nc.scalar.activation(
                    out=E[:, b * 4:(b + 1) * 4, :], in_=sps[:],
                    func=mybir.ActivationFunctionType.Exp, scale=1.0)
                # EM = E*mask ; z = rowsum(EM)
                for s4 in range(4):
                    s = b * 4 + s4
                    nc.vector.tensor_tensor(
                        out=EM[:, s, :], in0=E[:, s, :], in1=mask[:],
                        op=mybir.AluOpType.mult)
                nc.vector.tensor_reduce(
                    out=z[:, b * 4:(b + 1) * 4],
                    in_=EM[:, b * 4:(b + 1) * 4, :],
                    op=mybir.AluOpType.add, axis=mybir.AxisListType.X)
                nc.vector.reciprocal(out=zr[:, b * 4:(b + 1) * 4],
                                     in_=z[:, b * 4:(b + 1) * 4])
                for s4 in range(4):
                    s = b * 4 + s4
                    nc.vector.tensor_scalar(
                        out=P[:, s, :], in0=EM[:, s, :],
                        scalar1=zr[:, s:s + 1], scalar2=None,
                        op0=mybir.AluOpType.mult)
                # L = P^T (block-diagonal)
                ptp = ps_tr.tile([128, 4, 128], F16, tag="tr")
                for s4 in range(4):
                    s = b * 4 + s4
                    nc.tensor.transpose(ptp[:, s4, :], P[:, s, :], ident[:])
                nc.vector.tensor_copy(
                    out=L[:, b * 4:(b + 1) * 4, :], in_=ptp[:])
                # ctx^T[d, (t,h)] = Vb^T @ L per sub-tile
                cps = ps_s.tile([128, 4, 128], F32, tag="s")
                for s4 in range(4):
                    s = b * 4 + s4
                    nc.tensor.matmul(
                        cps[:, s4, :], Vb[:, s, :], L[:, s, :],
                        start=True, stop=True, skip_group_check=True)
                # per-head copy: src cols (s4, h*16+t) -> ctxT[d][h][64b:64b+64]
                for h in range(HEADS):
                    src = bass.AP(tensor=cps.tensor,
                                  offset=cps.offset + h * TS,
                                  ap=[cps.ap[0], [128, 4], [1, TS]])
                    if h % 2 == 0:
                        nc.scalar.copy(
                            out=ctxT[:, h, b * 64:(b + 1) * 64], in_=src)
                    else:
                        nc.vector.tensor_copy(
                            out=ctxT[:, h, b * 64:(b + 1) * 64], in_=src)
            return ctxT

        def emit_oproj(c, it, ctxT):
            tok0 = c * TC + it * TT
            for oh in range(2):
                pso = ps_o.tile([128, 512], F32, tag="o")
                for h in range(HEADS):
                    nc.tensor.matmul(
                        pso[:], ctxT[:, h, :],
                        wo_sb[:, h, oh * 512:(oh + 1) * 512],
                        start=(h == 0), stop=(h == HEADS - 1))
                osb = outsb_p.tile([128, 512], F16, tag="osb")
                nc.scalar.copy(out=osb[:], in_=pso[:])
                nc.sync.dma_start(
                    out_d[tok0:tok0 + TT, oh * 512:(oh + 1) * 512], osb[:])

        # ---------------- main schedule ----------------
        emit_loads(0)
        emit_loads(1)
        for it in range(TPC):
            emit_ln(0, it)

        for c in range(NCH):
            emit_loads(c + 2)
            emit_transposes(c)
            emit_projections(c)
            ctx_tiles = {}
            # stage C with next-chunk LN interleaved on DVE; O-proj lags
            # one tile so PE never waits on the ACT ctx copies.
            ctx_tiles[0] = emit_attention_tile(c, 0)
            emit_ln(c + 1, 0)
            ctx_tiles[1] = emit_attention_tile(c, 1)
            emit_ln(c + 1, 1)
            emit_oproj(c, 0, ctx_tiles.pop(0))
            ctx_tiles[2] = emit_attention_tile(c, 2)
            emit_ln(c + 1, 2)
            emit_oproj(c, 1, ctx_tiles.pop(1))
            ctx_tiles[3] = emit_attention_tile(c, 3)
            emit_ln(c + 1, 3)
            emit_oproj(c, 2, ctx_tiles.pop(2))
            emit_oproj(c, 3, ctx_tiles.pop(3))

    nc.finalize()
    return nc


def _host_mask():
    # h-major windows: row p = h*16+t, col q = g*16+t'; valid iff t == t'
    m = np.zeros((TT, TT), np.float32)
    p = np.arange(TT)
    m[p[:, None] % TS == p[None, :] % TS] = 1.0
    return m.astype(ml_dtypes.bfloat16)


def kernel(q, kv, gamma_m, beta_m, gamma_l, beta_l, Wq, Wkv, Wo):
    q = np.asarray(q, np.float32)
    kv = np.asarray(kv, np.float32)
    bs, patch, _ = q.shape
    T_total = bs * patch
    T_core = T_total // NCORES

    scale = DHEAD ** (-0.5)
    # fold LN gamma into the projection weights, beta into bias vectors
    wq_eff = (np.asarray(Wq, np.float32) * np.asarray(gamma_m, np.float32)[None, :]) * scale
    bq = (np.asarray(Wq, np.float32) @ np.asarray(beta_m, np.float32)) * scale
    wkv_eff = np.asarray(Wkv, np.float32) * np.asarray(gamma_l, np.float32)[None, :]
    bkv = np.asarray(Wkv, np.float32) @ np.asarray(beta_l, np.float32)
    with_bias_q = bool(np.any(bq != 0.0))
    with_bias_kv = bool(np.any(bkv != 0.0))

    # kernel weight layout: [in, out], fp16
    wq_t = np.ascontiguousarray(wq_eff.T).astype(np.float16)
    wkv_t = np.ascontiguousarray(wkv_eff.T).astype(np.float16)
    wo_t = np.ascontiguousarray(np.asarray(Wo, np.float32).T).astype(np.float16)
    mask = _host_mask()

    nc = build_nc(T_core, with_bias_q, with_bias_kv)

    qf = q.reshape(T_total, DIM).astype(np.float16)
    kvf = kv.reshape(T_total, DIM).astype(np.float16)
    in_maps = []
    for i in range(NCORES):
        m = {
            "q": np.ascontiguousarray(qf[i * T_core:(i + 1) * T_core]),
            "kv": np.ascontiguousarray(kvf[i * T_core:(i + 1) * T_core]),
            "wq": wq_t, "wkv": wkv_t, "wo": wo_t, "mask": mask,
        }
        if with_bias_q:
            m["bq"] = bq.reshape(1, DIM).astype(np.float16)
        if with_bias_kv:
            m["bkv"] = bkv.reshape(1, 2 * DIM).astype(np.float16)
        in_maps.append(m)

    res = run_bass_kernel_spmd(nc, in_maps, list(range(NCORES)))
    global LAST_RESULTS
    LAST_RESULTS = res
    out = np.concatenate(
        [np.asarray(res.results[i]["out"], np.float32) for i in range(NCORES)],
        axis=0)
    return out.reshape(bs, patch, DIM)


LAST_RESULTS = None
